# revision 10
# baseline (speedup 1.0000x reference)
"""Chamfer loss (single-term) Trainium2 Bass kernel.

Problem: B=8 batches of point clouds p1 [8192, 3], p2 [2048, 3]; loss =
(sum_n min_m ||p1_n - p2_m||^2 + sum_m min_n ||...||^2) / B.

Sharding: data-parallel over batch, one batch element per NeuronCore (8 cores).

Per-core algorithm. Define D'[n, m] = 2*<p1_n, p2_m> - |p2_m|^2, so
    min_m dist(n, m) = |p1_n|^2 - max_m D'[n, m]      (pass A)
and symmetrically for pass B with p1/p2 swapped. The |row|^2 sums are added
back on the host, so each pass only needs the row-maxes of D'.

D' comes from a K=5 fp16 matmul: lhsT rows [x, y, z, 1, 1] (stationary
[5, 128] slices), rhs rows [2x', 2y', 2z', -sq'_hi, -sq'_lo] where
sq' = |p'|^2 of the f16-rounded points split into fp16 hi+lo parts (exact to
~2^-21, so D' = -|p~1 - p~2|^2 + |p~1|^2 with f16-consistent points; the
rounding error at the min scales with the distance itself). fp16 matmuls are
~2.7x faster than fp32r and need no per-matmul weight reload cost (LDWEIGHTS
99ns vs 282ns). PSUM accumulates fp32. Four PE row-groups
(tile_position=(32c, 0)) run concurrently, one PSUM bank each.

Drain (HW-measured design): the old dual-stream max *scan* pays the DVE
recurrence (~2.33 cyc/position); plain tensor_tensor max has no recurrence
(1 cyc/position from PSUM) and pure-f16 TT gets the 2x_1P mode. Per 128-row
block (4 banks): ScalarE evacuates banks 1,3 to SBUF as f16; DVE computes
t0 = max(ps0, cp1), t1 = max(ps2, cp3) (mixed-dtype TT, 1x), then
t2 = max(t0, t1) (f16 TT, 2x) into one slot of a [128, 2, 512] pair buffer;
after every second block a single 3D tensor_reduce collapses the pair to two
row-max columns. f16 drain rounding adds ~1e-4 relative error (tolerance
2e-2).

The distance matrix never touches HBM. Host side reshapes/augments inputs
(O(N) prep) and combines 8 partial sums with the |row|^2 terms.
"""

import numpy as np
import ml_dtypes
from contextlib import ExitStack

import concourse.bass as bass
import concourse.bacc as bacc
import concourse.tile as tile
from concourse import mybir
from concourse.bass_utils import run_bass_kernel_spmd

F32 = mybir.dt.float32
F16 = mybir.dt.float16
BF16 = mybir.dt.bfloat16
X = mybir.AxisListType.X
XY = mybir.AxisListType.XY
MAX = mybir.AluOpType.max
ADD = mybir.AluOpType.add

N_FULL, M_FULL, B_FULL = 8192, 2048, 8
NEG_BIG = -3.0e38


def _chamfer_kernel(ctx, tc, y, inp, *, N, M, CN):
    """Emit the per-core kernel. See module docstring.

    inp: [20, N + M/4 + M + N/4] — per partition-group c (rows 5c..5c+5),
    columns are the concatenation of:
      la [N]:    [p1x, p1y, p1z, sq1, 1]              (same for all groups)
      ra [M/4]:  [2p2x, 2p2y, 2p2z, -1, -sq2][:, c*CM:(c+1)*CM]
      lb [M]:    [p2x, p2y, p2z, sq2, 1]              (same for all groups)
      rb [N/4]:  CN-chunks j = c, c+4, c+8, ... of [2p1x, 2p1y, 2p1z, -1, -sq1]
    y:  [128, 1]  per-partition partial sums (host adds them up).
    """
    nc = tc.nc
    NB = N // 128          # pass-A n-blocks (quads)
    MB = M // 128          # pass-B m-blocks
    CM = M // 4            # pass-A moving chunk (one PSUM bank)
    UB = N // (4 * CN)     # pass-B quads per m-block
    assert CM <= 512 and CN <= 512 and N % (4 * CN) == 0 and M % 128 == 0

    singles = ctx.enter_context(tc.tile_pool(name="singles", bufs=1))
    scp = ctx.enter_context(tc.tile_pool(name="scp", bufs=6))

    WTOT = N + CM + M + N // 4
    inp_s = singles.tile([128, WTOT], BF16)
    for c in range(4):
        p = 32 * c
        nc.sync.dma_start(out=inp_s[p:p + 5, :], in_=inp[5 * c:5 * c + 5, :])
    la_s = inp_s[:, 0:N]
    ra_s = inp_s[:, N:N + CM]
    lb_s = inp_s[:, N + CM:N + CM + M]
    rb_s = inp_s[:, N + CM + M:WTOT]

    rmA = singles.tile([128, NB], F32)
    rmB = singles.tile([128, MB, UB], F32)

    # Per block: TT-max tree instead of scans (TT has no recurrence: 1
    # cyc/pos from PSUM, and the pure-f16 level runs the 2x_1P mode).
    # ScalarE evacuates banks 1,3 as f16; DVE: t0=max(ps0,cp1),
    # t1=max(ps2,cp3) (mixed dtype, 1x), t2=max(t0,t1) (f16, 2x) into a
    # pair buffer; every second block one 3D tensor_reduce collapses both
    # blocks' [128,512] maxes to two row-max columns.
    pair_state = {}

    def do_quad(psum_pool, mk_matmul, w, out_pair, slot):
        assert w == 512
        if slot == 0:
            u_t = scp.tile([128, 2, 1024], F16, tag="u0")
            pair_state["u"] = u_t
        u = pair_state["u"]
        psA = psum_pool.tile([128, 1024], F32, tag="ps")
        mk_matmul(0, psA[:, 0:512])
        mk_matmul(1, psA[:, 512:1024])
        psB = psum_pool.tile([128, 1024], F32, tag="ps")
        mk_matmul(2, psB[:, 0:512])
        mk_matmul(3, psB[:, 512:1024])
        cpB = scp.tile([128, 1024], F16, tag="cp")
        nc.scalar.copy(out=cpB, in_=psB)
        nc.vector.tensor_tensor(u[:, slot, :], psA, cpB, MAX)
        if slot == 1:
            nc.vector.tensor_reduce(out=out_pair, in_=u, axis=X, op=MAX)

    with tc.tile_pool(name="psum", bufs=4, space="PSUM") as psum_pool:
        # Pass A: for each 128-row block of p1, stream all of p2 (as pairs).
        for b in range(NB):
            def mk_a(c, ps, b=b):
                p = 32 * c
                nc.tensor.matmul(
                    ps[:, :CM],
                    lhsT=la_s[p:p + 5, b * 128:(b + 1) * 128],
                    rhs=ra_s[p:p + 5, :],
                    start=True, stop=True,
                    tile_position=(p, 0),
                )
            do_quad(psum_pool, mk_a, CM,
                    rmA[:, (b // 2) * 2:(b // 2) * 2 + 2], b % 2)

        # Pass B: for each 128-row block of p2, stream all of p1 (as pairs).
        for mb in range(MB):
            for u in range(UB):
                def mk_b(c, ps, mb=mb, u=u):
                    p = 32 * c
                    nc.tensor.matmul(
                        ps[:, :CN],
                        lhsT=lb_s[p:p + 5, mb * 128:(mb + 1) * 128],
                        rhs=rb_s[p:p + 5, u * CN:(u + 1) * CN],
                        start=True, stop=True,
                        tile_position=(p, 0),
                    )
                do_quad(psum_pool, mk_b, CN,
                        rmB[:, mb, (u // 2) * 2:(u // 2) * 2 + 2], u % 2)

    # Combine: pass-B max over chunks, then sum everything.
    rmBm = singles.tile([128, MB], F32)
    nc.vector.tensor_reduce(out=rmBm, in_=rmB, axis=X, op=MAX)
    sA = singles.tile([128, 1], F32)
    sB = singles.tile([128, 1], F32)
    nc.vector.tensor_reduce(out=sA, in_=rmA, axis=X, op=ADD)
    nc.vector.tensor_reduce(out=sB, in_=rmBm, axis=X, op=ADD)
    colsum = singles.tile([128, 1], F32)
    nc.vector.tensor_add(colsum, sA, sB)
    # Final 128-value partition sum happens on the host (y is [128, 1]).
    nc.sync.dma_start(out=y, in_=colsum)


def build_module(N=N_FULL, M=M_FULL, CN=512):
    nc = bacc.Bacc("TRN2", target_bir_lowering=False, debug=False)
    WTOT = N + M // 4 + M + N // 4
    inp = nc.dram_tensor("inp", [20, WTOT], BF16, kind="ExternalInput").ap()
    y = nc.dram_tensor("y", [128, 1], F32, kind="ExternalOutput").ap()
    with tile.TileContext(nc) as tc:
        with ExitStack() as ctx:
            _chamfer_kernel(ctx, tc, y, inp, N=N, M=M, CN=CN)
    nc.compile()
    return nc


def make_core_inputs(p1, p2, CN=512):
    """Host-side layout/augmentation prep for one batch element."""
    p1 = np.asarray(p1, dtype=np.float32)
    p2 = np.asarray(p2, dtype=np.float32)
    N, M = p1.shape[0], p2.shape[0]
    sqsum = None  # set below; returned for the host-side combine
    # fp16 PE path: coordinates rounded to f16 (consistently on both sides);
    # D' = 2<p1,p2> - |p2|^2 with |p2|^2 carried as f16 hi+lo rows (exact to
    # ~2^-21); the row-side |p1|^2 is added back on the host (run()).
    bf16 = ml_dtypes.bfloat16
    p1h = p1.astype(bf16).astype(np.float32)
    p2h = p2.astype(bf16).astype(np.float32)
    sq1 = (p1h ** 2).sum(axis=1, dtype=np.float32)
    sq2 = (p2h ** 2).sum(axis=1, dtype=np.float32)
    sq1_hi = sq1.astype(bf16).astype(np.float32)
    sq1_lo = sq1 - sq1_hi
    sq2_hi = sq2.astype(bf16).astype(np.float32)
    sq2_lo = sq2 - sq2_hi
    onesN = np.ones(N, np.float32)
    onesM = np.ones(M, np.float32)

    A_l = np.stack([p1h[:, 0], p1h[:, 1], p1h[:, 2], onesN, onesN], 0)  # [5, N]
    A_r = np.stack([2 * p2h[:, 0], 2 * p2h[:, 1], 2 * p2h[:, 2],
                    -sq2_hi, -sq2_lo], 0)
    B_l = np.stack([p2h[:, 0], p2h[:, 1], p2h[:, 2], onesM, onesM], 0)  # [5, M]
    B_r = np.stack([2 * p1h[:, 0], 2 * p1h[:, 1], 2 * p1h[:, 2],
                    -sq1_hi, -sq1_lo], 0)

    CM = M // 4
    nch = N // CN
    rows = []
    for c in range(4):
        ra_c = A_r[:, c * CM:(c + 1) * CM]
        rb_c = np.concatenate(
            [B_r[:, j * CN:(j + 1) * CN] for j in range(c, nch, 4)], 1)
        rows.append(np.concatenate([A_l, ra_c, B_l, rb_c], 1))
    sqsum = np.float64(sq1.sum(dtype=np.float64) + sq2.sum(dtype=np.float64))
    full = np.ascontiguousarray(
        np.concatenate(rows, 0).astype(ml_dtypes.bfloat16))
    return {"inp": full}, sqsum


_MODULE_CACHE = {}


def _get_module(key, **kw):
    if key not in _MODULE_CACHE:
        _MODULE_CACHE[key] = build_module(**kw)
    return _MODULE_CACHE[key]


def run(inputs, trace=False):
    """Run the full-size problem on 8 cores. Returns (result, BassKernelResults)."""
    gt = np.asarray(inputs["gt_points"], dtype=np.float32)
    sp = np.asarray(inputs["structure_points"], dtype=np.float32)
    B = gt.shape[0]
    assert B == B_FULL and gt.shape[1] == N_FULL and sp.shape[1] == M_FULL
    prepped = [make_core_inputs(gt[b], sp[b]) for b in range(B)]
    in_maps = [p[0] for p in prepped]
    sqsums = [p[1] for p in prepped]
    nc = _get_module(("full",))
    res = run_bass_kernel_spmd(nc, in_maps, list(range(B)), trace=trace)
    total = np.float64(0.0)
    for b, r in enumerate(res.results):
        total += sqsums[b] - np.float64(r["y"].sum(dtype=np.float64))
    return np.float32(total / B_FULL), res


def kernel(**inputs):
    return run(inputs)[0]



# revision 11
# speedup vs baseline: 1.0738x; 1.0738x over previous
"""Chamfer loss (single-term) Trainium2 Bass kernel.

Problem: B=8 batches of point clouds p1 [8192, 3], p2 [2048, 3]; loss =
(sum_n min_m ||p1_n - p2_m||^2 + sum_m min_n ||...||^2) / B.

Sharding: data-parallel over batch, one batch element per NeuronCore (8 cores).

Per-core algorithm. Define D'[n, m] = 2*<p1_n, p2_m> - |p2_m|^2, so
    min_m dist(n, m) = |p1_n|^2 - max_m D'[n, m]      (pass A)
and symmetrically for pass B with p1/p2 swapped. The |row|^2 sums are added
back on the host, so each pass only needs the row-maxes of D'.

D' comes from a K=5 fp16 matmul: lhsT rows [x, y, z, 1, 1] (stationary
[5, 128] slices), rhs rows [2x', 2y', 2z', -sq'_hi, -sq'_lo] where
sq' = |p'|^2 of the f16-rounded points split into fp16 hi+lo parts (exact to
~2^-21, so D' = -|p~1 - p~2|^2 + |p~1|^2 with f16-consistent points; the
rounding error at the min scales with the distance itself). fp16 matmuls are
~2.7x faster than fp32r and need no per-matmul weight reload cost (LDWEIGHTS
99ns vs 282ns). PSUM accumulates fp32. Four PE row-groups
(tile_position=(32c, 0)) run concurrently, one PSUM bank each.

Drain (HW-measured design): the old dual-stream max *scan* pays the DVE
recurrence (~2.33 cyc/position); plain tensor_tensor max has no recurrence
(1 cyc/position from PSUM) and pure-f16 TT gets the 2x_1P mode. Per 128-row
block (4 banks): ScalarE evacuates banks 1,3 to SBUF as f16; DVE computes
t0 = max(ps0, cp1), t1 = max(ps2, cp3) (mixed-dtype TT, 1x), then
t2 = max(t0, t1) (f16 TT, 2x) into one slot of a [128, 2, 512] pair buffer;
after every second block a single 3D tensor_reduce collapses the pair to two
row-max columns. f16 drain rounding adds ~1e-4 relative error (tolerance
2e-2).

The distance matrix never touches HBM. Host side reshapes/augments inputs
(O(N) prep) and combines 8 partial sums with the |row|^2 terms.
"""

import numpy as np
import ml_dtypes
from contextlib import ExitStack

import concourse.bass as bass
import concourse.bacc as bacc
import concourse.tile as tile
from concourse import mybir
from concourse.bass_utils import run_bass_kernel_spmd

F32 = mybir.dt.float32
F16 = mybir.dt.float16
BF16 = mybir.dt.bfloat16
X = mybir.AxisListType.X
XY = mybir.AxisListType.XY
MAX = mybir.AluOpType.max
ADD = mybir.AluOpType.add

N_FULL, M_FULL, B_FULL = 8192, 2048, 8
NEG_BIG = -3.0e38


def _chamfer_kernel(ctx, tc, y, inp, *, N, M, CN):
    """Emit the per-core kernel. See module docstring.

    inp: [20, N + M/4 + M + N/4] — per partition-group c (rows 5c..5c+5),
    columns are the concatenation of:
      la [N]:    [p1x, p1y, p1z, sq1, 1]              (same for all groups)
      ra [M/4]:  [2p2x, 2p2y, 2p2z, -1, -sq2][:, c*CM:(c+1)*CM]
      lb [M]:    [p2x, p2y, p2z, sq2, 1]              (same for all groups)
      rb [N/4]:  CN-chunks j = c, c+4, c+8, ... of [2p1x, 2p1y, 2p1z, -1, -sq1]
    y:  [128, 1]  per-partition partial sums (host adds them up).
    """
    nc = tc.nc
    NB = N // 128          # pass-A n-blocks (quads)
    MB = M // 128          # pass-B m-blocks
    CM = M // 4            # pass-A moving chunk (one PSUM bank)
    UB = N // (4 * CN)     # pass-B quads per m-block
    assert CM <= 512 and CN <= 512 and N % (4 * CN) == 0 and M % 128 == 0

    singles = ctx.enter_context(tc.tile_pool(name="singles", bufs=1))
    scp = ctx.enter_context(tc.tile_pool(name="scp", bufs=6))

    WTOT = N + CM + M + N // 4
    inp_s = singles.tile([128, WTOT], BF16)
    for c in range(4):
        p = 32 * c
        nc.sync.dma_start(out=inp_s[p:p + 5, :], in_=inp[5 * c:5 * c + 5, :])
    la_s = inp_s[:, 0:N]
    ra_s = inp_s[:, N:N + CM]
    lb_s = inp_s[:, N + CM:N + CM + M]
    rb_s = inp_s[:, N + CM + M:WTOT]

    rmA = singles.tile([128, NB], F32)
    rmB = singles.tile([128, MB, UB], F32)

    # Per block: TT-max tree instead of scans (TT has no recurrence: 1
    # cyc/pos from PSUM, and the pure-f16 level runs the 2x_1P mode).
    # ScalarE evacuates banks 1,3 as f16; DVE: t0=max(ps0,cp1),
    # t1=max(ps2,cp3) (mixed dtype, 1x), t2=max(t0,t1) (f16, 2x) into a
    # pair buffer; every second block one 3D tensor_reduce collapses both
    # blocks' [128,512] maxes to two row-max columns.
    pair_state = {}

    def do_quad(psum_pool, mk_matmul, w, out_pair, slot):
        assert w == 512
        if slot == 0:
            u0_t = scp.tile([128, 2, 512], F16, tag="u0")
            u1_t = scp.tile([128, 2, 512], F16, tag="u1")
            pair_state["u0"], pair_state["u1"] = u0_t, u1_t
        u0, u1 = pair_state["u0"], pair_state["u1"]
        ps0 = psum_pool.tile([128, 512], F32, tag="ps")
        mk_matmul(0, ps0)
        ps1 = psum_pool.tile([128, 512], F32, tag="ps")
        mk_matmul(1, ps1)
        cp1 = scp.tile([128, 512], F16, tag="cp")
        nc.scalar.copy(out=cp1, in_=ps1)
        nc.vector.tensor_tensor(u0[:, slot, :], ps0, cp1, MAX)
        ps2 = psum_pool.tile([128, 512], F32, tag="ps")
        mk_matmul(2, ps2)
        ps3 = psum_pool.tile([128, 512], F32, tag="ps")
        mk_matmul(3, ps3)
        cp3 = scp.tile([128, 512], F16, tag="cp")
        nc.scalar.copy(out=cp3, in_=ps3)
        nc.vector.tensor_tensor(u1[:, slot, :], ps2, cp3, MAX)
        if slot == 1:
            pair = scp.tile([128, 2, 512], F16, tag="pair")
            nc.vector.tensor_tensor(pair, u0, u1, MAX)
            nc.vector.tensor_reduce(out=out_pair, in_=pair, axis=X, op=MAX)

    with tc.tile_pool(name="psum", bufs=8, space="PSUM") as psum_pool:
        # Pass A: for each 128-row block of p1, stream all of p2 (as pairs).
        for b in range(NB):
            def mk_a(c, ps, b=b):
                p = 32 * c
                nc.tensor.matmul(
                    ps[:, :CM],
                    lhsT=la_s[p:p + 5, b * 128:(b + 1) * 128],
                    rhs=ra_s[p:p + 5, :],
                    start=True, stop=True,
                    tile_position=(p, 0),
                )
            do_quad(psum_pool, mk_a, CM,
                    rmA[:, (b // 2) * 2:(b // 2) * 2 + 2], b % 2)

        # Pass B: for each 128-row block of p2, stream all of p1 (as pairs).
        for mb in range(MB):
            for u in range(UB):
                def mk_b(c, ps, mb=mb, u=u):
                    p = 32 * c
                    nc.tensor.matmul(
                        ps[:, :CN],
                        lhsT=lb_s[p:p + 5, mb * 128:(mb + 1) * 128],
                        rhs=rb_s[p:p + 5, u * CN:(u + 1) * CN],
                        start=True, stop=True,
                        tile_position=(p, 0),
                    )
                do_quad(psum_pool, mk_b, CN,
                        rmB[:, mb, (u // 2) * 2:(u // 2) * 2 + 2], u % 2)

    # Combine: pass-B max over chunks, then sum everything.
    rmBm = singles.tile([128, MB], F32)
    nc.vector.tensor_reduce(out=rmBm, in_=rmB, axis=X, op=MAX)
    sA = singles.tile([128, 1], F32)
    sB = singles.tile([128, 1], F32)
    nc.vector.tensor_reduce(out=sA, in_=rmA, axis=X, op=ADD)
    nc.vector.tensor_reduce(out=sB, in_=rmBm, axis=X, op=ADD)
    colsum = singles.tile([128, 1], F32)
    nc.vector.tensor_add(colsum, sA, sB)
    # Final 128-value partition sum happens on the host (y is [128, 1]).
    nc.sync.dma_start(out=y, in_=colsum)


def build_module(N=N_FULL, M=M_FULL, CN=512):
    nc = bacc.Bacc("TRN2", target_bir_lowering=False, debug=False)
    WTOT = N + M // 4 + M + N // 4
    inp = nc.dram_tensor("inp", [20, WTOT], BF16, kind="ExternalInput").ap()
    y = nc.dram_tensor("y", [128, 1], F32, kind="ExternalOutput").ap()
    with tile.TileContext(nc) as tc:
        with ExitStack() as ctx:
            _chamfer_kernel(ctx, tc, y, inp, N=N, M=M, CN=CN)
    nc.compile()
    return nc


def make_core_inputs(p1, p2, CN=512):
    """Host-side layout/augmentation prep for one batch element."""
    p1 = np.asarray(p1, dtype=np.float32)
    p2 = np.asarray(p2, dtype=np.float32)
    N, M = p1.shape[0], p2.shape[0]
    sqsum = None  # set below; returned for the host-side combine
    # fp16 PE path: coordinates rounded to f16 (consistently on both sides);
    # D' = 2<p1,p2> - |p2|^2 with |p2|^2 carried as f16 hi+lo rows (exact to
    # ~2^-21); the row-side |p1|^2 is added back on the host (run()).
    bf16 = ml_dtypes.bfloat16
    p1h = p1.astype(bf16).astype(np.float32)
    p2h = p2.astype(bf16).astype(np.float32)
    sq1 = (p1h ** 2).sum(axis=1, dtype=np.float32)
    sq2 = (p2h ** 2).sum(axis=1, dtype=np.float32)
    sq1_hi = sq1.astype(bf16).astype(np.float32)
    sq1_lo = sq1 - sq1_hi
    sq2_hi = sq2.astype(bf16).astype(np.float32)
    sq2_lo = sq2 - sq2_hi
    onesN = np.ones(N, np.float32)
    onesM = np.ones(M, np.float32)

    A_l = np.stack([p1h[:, 0], p1h[:, 1], p1h[:, 2], onesN, onesN], 0)  # [5, N]
    A_r = np.stack([2 * p2h[:, 0], 2 * p2h[:, 1], 2 * p2h[:, 2],
                    -sq2_hi, -sq2_lo], 0)
    B_l = np.stack([p2h[:, 0], p2h[:, 1], p2h[:, 2], onesM, onesM], 0)  # [5, M]
    B_r = np.stack([2 * p1h[:, 0], 2 * p1h[:, 1], 2 * p1h[:, 2],
                    -sq1_hi, -sq1_lo], 0)

    CM = M // 4
    nch = N // CN
    rows = []
    for c in range(4):
        ra_c = A_r[:, c * CM:(c + 1) * CM]
        rb_c = np.concatenate(
            [B_r[:, j * CN:(j + 1) * CN] for j in range(c, nch, 4)], 1)
        rows.append(np.concatenate([A_l, ra_c, B_l, rb_c], 1))
    sqsum = np.float64(sq1.sum(dtype=np.float64) + sq2.sum(dtype=np.float64))
    full = np.ascontiguousarray(
        np.concatenate(rows, 0).astype(ml_dtypes.bfloat16))
    return {"inp": full}, sqsum


_MODULE_CACHE = {}


def _get_module(key, **kw):
    if key not in _MODULE_CACHE:
        _MODULE_CACHE[key] = build_module(**kw)
    return _MODULE_CACHE[key]


def run(inputs, trace=False):
    """Run the full-size problem on 8 cores. Returns (result, BassKernelResults)."""
    gt = np.asarray(inputs["gt_points"], dtype=np.float32)
    sp = np.asarray(inputs["structure_points"], dtype=np.float32)
    B = gt.shape[0]
    assert B == B_FULL and gt.shape[1] == N_FULL and sp.shape[1] == M_FULL
    prepped = [make_core_inputs(gt[b], sp[b]) for b in range(B)]
    in_maps = [p[0] for p in prepped]
    sqsums = [p[1] for p in prepped]
    nc = _get_module(("full",))
    res = run_bass_kernel_spmd(nc, in_maps, list(range(B)), trace=trace)
    total = np.float64(0.0)
    for b, r in enumerate(res.results):
        total += sqsums[b] - np.float64(r["y"].sum(dtype=np.float64))
    return np.float32(total / B_FULL), res


def kernel(**inputs):
    return run(inputs)[0]

